# revision 1
# baseline (speedup 1.0000x reference)
"""Multi-head causal self-attention (B=4, S=2048, D=1024, H=16) on 8 trn2 cores.

Sharding: core c = (batch b = c//2, head-group g = c%2 of 8 heads).
Each core computes Q/K/V projections for its 8 heads over its batch's 2048
tokens, causal attention, and a PARTIAL output projection over its 512
feature dims. The host adds the two partial [2048, 1024] outputs per batch.
No on-device collectives.

On-core dataflow (all f32):
  QT[e,t], KT[e,t] = W @ x^T          (e on partitions -> head-dim-major)
  V'[t, h, 0:64] = x @ Wv^T, col 64 = 1.0   (ones column makes PV matmul
                                             also produce the softmax sum Z)
  St[s,q] = K^T-slices .T @ Q^T-slices      (scores transposed; K=64 row-
                                             packed pairs of heads)
  E = exp(St/8) via ACT from PSUM; causal mask pre-added to the scores
      PSUM as -1e9 via an identity-matmul of a precomputed mask (exact)
  numerT[dv,q], Z[q] = V'^T.T @ E            (accumulated over s-tiles)
  attnT = numerT * (1/Z broadcast)
  out[t,e] = attnT-slices .T @ Wo^T-slices   (partial over this core's dims)
"""

import numpy as np

import concourse.bass as bass
import concourse.mybir as mybir
import concourse.tile as tile
from concourse import bacc
from concourse.bass_utils import run_bass_kernel_spmd

F32 = mybir.dt.float32
F32R = mybir.dt.float32r  # PE full-rate fp32 mode (N>=256)
MM_DT = F32R
AF = mybir.ActivationFunctionType

B = 4
S = 2048
D_MODEL = 1024
E = 512          # feature dims per core (8 heads x 64)
HEADS = 8        # heads per core
DK = 64
NQ = 4           # 512-token q-slices
ND = 8           # 128-dim d_model tiles
NT = 16          # 128-token tiles
SCALE = 0.125    # 1/sqrt(dk)


# phase switches for cost-model bisection (always all-True in production)
DO_QKV = True
DO_ATTN = True
DO_WO = True
DO_EXP = True   # False: skip exp (implies DO_PV=False)
DO_PV = True    # False: skip PV matmuls + normalization


def _emit(tc, xT, wqT, wkT, wvT, woT, out):
    nc = tc.nc
    with tc.tile_pool(name="singles", bufs=1) as singles:
        qt = [singles.tile([128, S], MM_DT, name=f"qt{i}") for i in range(4)]
        kt = [singles.tile([128, S], MM_DT, name=f"kt{i}") for i in range(4)]
        vt = singles.tile([128, NT, HEADS, DK + 1], MM_DT, name="vt")
        wot = singles.tile([128, 4, D_MODEL], MM_DT, name="wot")

        ident = singles.tile([128, 128], MM_DT, name="ident")
        cmask = singles.tile([128, 4, 512], MM_DT, name="cmask")

        nc.sync.dma_start(out=wot, in_=woT)
        with tc.tile_pool(name="scratch", bufs=1) as scratch:
            sc = scratch.tile([128, 4, 512], F32, name="sc")
            nc.vector.memset(sc[:, 0, 0:128], 1.0)
            nc.vector.tensor_copy(vt[:, :, :, DK:DK + 1],
                                  sc[:, 0, 0:128].rearrange("p (a b) -> p a b", a=NT))
            # identity: keep 1.0 on the diagonal, 0 elsewhere
            nc.gpsimd.memset(sc[:, 0, 0:128], 0.0)
            nc.gpsimd.affine_select(
                out=sc[:, 0, 0:128], in_=sc[:, 0, 0:128], pattern=[[-1, 128]],
                compare_op=mybir.AluOpType.not_equal, fill=1.0,
                base=0, channel_multiplier=1)
            nc.vector.tensor_copy(ident, sc[:, 0, 0:128])
            # causal masks m_r[s, q] = 0 where s <= q - 128r else -1e9
            nc.gpsimd.memset(sc, 0.0)
            for r in range(4):
                nc.gpsimd.affine_select(
                    out=sc[:, r, :], in_=sc[:, r, :], pattern=[[1, 512]],
                    compare_op=mybir.AluOpType.is_ge, fill=-1e9,
                    base=-128 * r, channel_multiplier=-1)
            nc.vector.tensor_copy(cmask, sc)

        # ---------------- Phase 1: Q/K/V projections ----------------
        with (
            tc.tile_pool(name="wpool", bufs=1) as wpool,
            tc.tile_pool(name="xtc", bufs=9) as xtp,
            tc.tile_pool(name="qkps", bufs=4, space="PSUM") as qkps,
        ):
            wq = wpool.tile([128, ND, E], MM_DT, name="wq")
            wk = wpool.tile([128, ND, E], MM_DT, name="wk")
            wv = wpool.tile([128, ND, E], MM_DT, name="wv")
            nc.sync.dma_start(out=wq, in_=wqT)
            nc.sync.dma_start(out=wk, in_=wkT)
            nc.sync.dma_start(out=wv, in_=wvT)
            for j in range(NQ if DO_QKV else 0):
                tsl = slice(j * 512, (j + 1) * 512)
                chunks = []
                for d in range(ND):
                    ck = xtp.tile([128, 512], MM_DT, tag="xc", name=f"xc{j}_{d}")
                    nc.sync.dma_start(out=ck, in_=xT[j, d])
                    chunks.append(ck)
                for w, dst in ((wq, qt), (wk, kt)):
                    for e in range(4):
                        esl = slice(e * 128, (e + 1) * 128)
                        ps = qkps.tile([128, 512], F32, tag="qk", name=f"ps{j}_{e}")
                        for d in range(ND):
                            nc.tensor.matmul(ps, w[:, d, esl], chunks[d],
                                             start=(d == 0), stop=(d == ND - 1))
                        nc.vector.tensor_copy(dst[e][:, tsl], ps)
                for u in range(4):
                    i = 4 * j + u
                    usl = slice(u * 128, (u + 1) * 128)
                    ps = qkps.tile([128, 512], F32, tag="qk", name=f"psv{j}_{u}")
                    for d in range(ND):
                        nc.tensor.matmul(ps, chunks[d][:, usl], wv[:, d, :],
                                         start=(d == 0), stop=(d == ND - 1))
                    nc.vector.tensor_copy(
                        vt[:, i, :, 0:DK],
                        ps.rearrange("p (h k) -> p h k", h=HEADS))

        # ------------- Phase 2+3: attention + output projection -------------
        with (
            tc.tile_pool(name="stps", bufs=3, space="PSUM") as stps,
            tc.tile_pool(name="smps", bufs=2, space="PSUM") as smps,
            tc.tile_pool(name="expp", bufs=6) as expp,
            tc.tile_pool(name="attp", bufs=8) as attp,
            tc.tile_pool(name="nrmp", bufs=2) as nrmp,
            tc.tile_pool(name="outp", bufs=2) as outp,
        ):
            for j in range(NQ if (DO_ATTN or DO_WO) else 0):
                qsl = slice(j * 512, (j + 1) * 512)
                att_j = [attp.tile([128, 512], MM_DT, tag="at", name=f"at{j}_{f}")
                         for f in range(4)]
                for hp in range(4 if DO_ATTN else 0):
                    hA, hB = 2 * hp, 2 * hp + 1
                    if DO_PV:
                        pvA = smps.tile([DK + 1, 512], F32, tag="sp", name=f"pvA{j}{hp}")
                        pvB = smps.tile([DK + 1, 512], F32, tag="sp", name=f"pvB{j}{hp}")
                    nst = 4 * (j + 1)
                    for g in range(nst // 2):
                        stA = stps.tile([128, 2, 512], F32, tag="st", name=f"stA{j}{hp}{g}")
                        stB = stps.tile([128, 2, 512], F32, tag="st", name=f"stB{j}{hp}{g}")
                        for u in range(2):
                            i = 2 * g + u
                            r = i - 4 * j
                            ssl = slice(i * 128, (i + 1) * 128)
                            diag = r >= 0
                            # diag tiles: scores only over the valid q-range
                            # [128r, 512); the full-width mask matmul then
                            # overwrites the unwritten region (has_written=0)
                            # with -1e9 and adds exact 0.0 on the valid part.
                            q0 = 128 * r if diag else 0
                            qv = slice(j * 512 + q0, (j + 1) * 512)
                            if diag:
                                # mask first (full width, initializes bank),
                                # then scores accumulate over the valid range
                                nc.tensor.matmul(stA[:, u, :], ident,
                                                 cmask[:, r, :], start=True,
                                                 stop=False)
                                nc.tensor.matmul(stB[:, u, :], ident,
                                                 cmask[:, r, :], start=True,
                                                 stop=False)
                            nc.tensor.matmul(stA[:, u, q0:], kt[hp][0:64, ssl],
                                             qt[hp][0:64, qv], start=not diag,
                                             stop=True, tile_position=(0, 0))
                            nc.tensor.matmul(stB[:, u, q0:], kt[hp][64:128, ssl],
                                             qt[hp][64:128, qv], start=not diag,
                                             stop=True, tile_position=(64, 0))
                        if not DO_EXP:
                            continue
                        exA = expp.tile([128, 2, 512], MM_DT, tag="ex", name=f"exA{j}{hp}{g}")
                        exB = expp.tile([128, 2, 512], MM_DT, tag="ex", name=f"exB{j}{hp}{g}")
                        nc.scalar.activation(exA, stA, AF.Exp, scale=SCALE)
                        nc.scalar.activation(exB, stB, AF.Exp, scale=SCALE)
                        if not DO_PV:
                            continue
                        for u in range(2):
                            i = 2 * g + u
                            nc.tensor.matmul(pvA, vt[:, i, hA, :], exA[:, u, :],
                                             start=(i == 0), stop=(i == nst - 1))
                            nc.tensor.matmul(pvB, vt[:, i, hB, :], exB[:, u, :],
                                             start=(i == 0), stop=(i == nst - 1))
                    for pvx, p0 in (((pvA, 0), (pvB, 64)) if DO_PV else ()):
                        rz = nrmp.tile([1, 512], F32, tag="rz", name=f"rz{j}{hp}{p0}")
                        bz = nrmp.tile([64, 512], F32, tag="bz", name=f"bz{j}{hp}{p0}")
                        nc.vector.reciprocal(rz, pvx[DK:DK + 1, :])
                        nc.gpsimd.partition_broadcast(bz, rz)
                        nc.vector.tensor_mul(att_j[hp][p0:p0 + 64, :],
                                             pvx[0:DK, :], bz)
                for tt in range(4 if DO_WO else 0):
                    ot = outp.tile([128, D_MODEL], F32, tag="ot", name=f"ot{j}{tt}")
                    ttsl = slice(tt * 128, (tt + 1) * 128)
                    for eo in range(2):
                        pso = smps.tile([128, 512], F32, tag="sp", name=f"po{j}{tt}{eo}")
                        for f in range(4):
                            nc.tensor.matmul(pso, att_j[f][:, ttsl],
                                             wot[:, f, eo * 512:(eo + 1) * 512],
                                             start=(f == 0), stop=(f == 3))
                        nc.vector.tensor_copy(ot[:, eo * 512:(eo + 1) * 512], pso)
                    t0 = j * 512 + tt * 128
                    nc.sync.dma_start(out=out[t0:t0 + 128, :], in_=ot)


def build_nc(reps=1):
    nc = bacc.Bacc(None, target_bir_lowering=False, debug=False)
    xT = nc.dram_tensor("xT", [NQ, ND, 128, 512], MM_DT, kind="ExternalInput")
    wqT = nc.dram_tensor("wqT", [128, ND, E], MM_DT, kind="ExternalInput")
    wkT = nc.dram_tensor("wkT", [128, ND, E], MM_DT, kind="ExternalInput")
    wvT = nc.dram_tensor("wvT", [128, ND, E], MM_DT, kind="ExternalInput")
    woT = nc.dram_tensor("woT", [128, 4, D_MODEL], MM_DT, kind="ExternalInput")
    out = nc.dram_tensor("out", [S, D_MODEL], F32, kind="ExternalOutput")
    aps = (xT.ap(), wqT.ap(), wkT.ap(), wvT.ap(), woT.ap(), out.ap())
    with tile.TileContext(nc) as tc:
        if reps == 1:
            _emit(tc, *aps)
        else:
            with tc.For_i(0, reps, 1):
                _emit(tc, *aps)
    nc.compile()
    return nc


def make_in_maps(x, W_Q, W_K, W_V, W_O):
    in_maps = []
    for c in range(8):
        b, g = divmod(c, 2)
        sl = slice(g * E, (g + 1) * E)
        xt = x[b].T  # [D, S]
        xt4 = np.ascontiguousarray(
            xt.reshape(8, 128, 4, 512).transpose(2, 0, 1, 3))  # [j, d, 128, 512]
        def wtile(w):  # [D, E] -> [128, 8, E]
            return np.ascontiguousarray(w.reshape(8, 128, -1).transpose(1, 0, 2))
        in_maps.append({
            "xT": xt4,
            "wqT": wtile(W_Q[sl, :].T),
            "wkT": wtile(W_K[sl, :].T),
            "wvT": wtile(W_V[sl, :].T),
            "woT": np.ascontiguousarray(
                W_O[:, sl].T.reshape(4, 128, 1024).transpose(1, 0, 2)),
        })
    return in_maps


_NC_CACHE = None


def kernel(x, W_Q, W_K, W_V, W_O, _trace=False):
    global _NC_CACHE
    if _NC_CACHE is None:
        _NC_CACHE = build_nc()
    nc = _NC_CACHE
    in_maps = make_in_maps(x, W_Q, W_K, W_V, W_O)
    r = run_bass_kernel_spmd(nc, in_maps, list(range(8)), trace=_trace)
    kernel.last_result = r
    out = np.empty((B, S, D_MODEL), np.float32)
    for b in range(B):
        out[b] = r.results[2 * b]["out"] + r.results[2 * b + 1]["out"]
    return out



# revision 9
# speedup vs baseline: 1.0599x; 1.0599x over previous
"""Multi-head causal self-attention (B=4, S=2048, D=1024, H=16) on 8 trn2 cores.

Sharding: core c = (batch b = c//2, head-group g = c%2 of 8 heads).
Each core computes Q/K/V projections for its 8 heads over its batch's 2048
tokens, causal attention, and a PARTIAL output projection over its 512
feature dims. The host adds the two partial [2048, 1024] outputs per batch.
No on-device collectives.

vs the original f32r kernel:
  - Q/K/V projections run as 3-pass fp8e4m3 DoubleRow (W,x split hi+lo on
    host, lo*lo term dropped; W pre-scaled x32 to clear e4m3 subnormals,
    compensated in the exp scale / W_O). 256-contraction per instruction at
    0.5 cyc/row: 25% fewer PE cycles than f32r, error ~1e-3.
  - Scores, exp output E, and PV run in bf16 (same PE rate as f32r, half
    the SBUF, 2x DVE copies).
  - The causal mask costs ZERO PE work: score tiles are trimmed to the
    pair-level causal window (clamped to N>=256 to keep full PE rate) and
    the sub-diagonal triangle of E is zeroed by gpsimd affine_select.
  - exp scale folds 1/sqrt(dk), the x32^2 weight scale, and a -1 bias
    (headroom; cancels in the softmax ratio).
  - A ones-column in V makes each PV matmul also emit the softmax
    denominator Z.
  - Projections, attention and W_O are emitted per 512-token q-slice in one
    loop so ACT exp work overlaps PE matmuls across slices.
"""

import ml_dtypes
import numpy as np

import concourse.bass as bass
import concourse.mybir as mybir
import concourse.tile as tile
from concourse import bacc
from concourse.bass_utils import run_bass_kernel_spmd

F32 = mybir.dt.float32
F32R = mybir.dt.float32r
F8 = mybir.dt.float8e4
BF16 = mybir.dt.bfloat16
AF = mybir.ActivationFunctionType
DR = mybir.MatmulPerfMode.DoubleRow
E4 = ml_dtypes.float8_e4m3fn

B = 4
S = 2048
D_MODEL = 1024
E = 512          # feature dims per core (8 heads x 64)
HEADS = 8        # heads per core
DK = 64
NQ = 4           # 512-token q-slices
NKK = 4          # 256-dim d_model double-tiles
NT = 16          # 128-token tiles
W_SCALE = 32.0   # host pre-scale of W_Q/W_K/W_V before fp8 quantization
SCALE = 0.125 / (W_SCALE * W_SCALE)
EBIAS = -1.0     # exp headroom shift; cancels in softmax ratio


def _emit(tc, x8h, x8l, wq8hd, wq8ld, wk8hd, wk8ld, wv8hd, wv8ld, woT, out):
    nc = tc.nc
    with tc.tile_pool(name="singles", bufs=1) as singles, \
         tc.tile_pool(name="wpool", bufs=2) as wpool:
        wq = [wpool.tile([128, NKK, 2, E], F8, tag=f"wq{p}", name=f"wq{p}") for p in range(2)]
        wk = [wpool.tile([128, NKK, 2, E], F8, tag=f"wk{p}", name=f"wk{p}") for p in range(2)]
        wv = [wpool.tile([128, NKK, 2, E], F8, tag=f"wv{p}", name=f"wv{p}") for p in range(2)]
        wot = wpool.tile([128, 4, D_MODEL], F32R, tag="wot", name="wot")
        for t, d in ((wq[0], wq8hd), (wq[1], wq8ld), (wk[0], wk8hd),
                     (wk[1], wk8ld), (wv[0], wv8hd), (wv[1], wv8ld),
                     (wot, woT)):
            nc.sync.dma_start(out=t, in_=d)
        # K/Q with dk on partitions, head pair hp = (2hp, 2hp+1): [128, S]
        qt = [singles.tile([128, S], BF16, name=f"qt{hp}") for hp in range(4)]
        kt = [singles.tile([128, S], BF16, name=f"kt{hp}") for hp in range(4)]
        # V + ones column: [tok 128, s-tile, head, dk+1]
        vt = singles.tile([128, NT, HEADS, DK + 1], BF16, name="vt")
        nc.gpsimd.memset(vt[:, :, :, DK:DK + 1], 1.0)
        ebias = singles.tile([128, 1], F32, name="ebias")
        nc.gpsimd.memset(ebias, EBIAS)

        with (
            tc.tile_pool(name="xp", bufs=16) as xp,
            tc.tile_pool(name="pp", bufs=2, space="PSUM") as pp,
            tc.tile_pool(name="stp", bufs=2, space="PSUM") as stp,
            tc.tile_pool(name="pvp", bufs=2, space="PSUM") as pvp,
            tc.tile_pool(name="exq", bufs=6) as exq,
            tc.tile_pool(name="attp", bufs=8) as attp,
            tc.tile_pool(name="nrmp", bufs=4) as nrmp,
            tc.tile_pool(name="outp", bufs=3) as outp,
        ):
            def emit_wo(jw, att_w):
                for tt in range(4):
                    ot = outp.tile([128, D_MODEL], F32, tag="ot",
                                   name=f"ot{jw}{tt}")
                    ttsl = slice(tt * 128, (tt + 1) * 128)
                    for eo in range(2):
                        pso = pp.tile([128, 512], F32, tag="pp",
                                      name=f"po{jw}{tt}{eo}")
                        for f in range(4):
                            nc.tensor.matmul(pso, att_w[f][:, ttsl],
                                             wot[:, f, eo * 512:(eo + 1) * 512],
                                             start=(f == 0), stop=(f == 3))
                        nc.vector.tensor_copy(ot[:, eo * 512:(eo + 1) * 512],
                                              pso)
                    t0 = jw * 512 + tt * 128
                    nc.sync.dma_start(out=out[t0:t0 + 128, :], in_=ot)

            prev_att = None
            for j in range(NQ):
                tsl = slice(j * 512, (j + 1) * 512)
                xh = []
                xl = []
                for kk in range(NKK):
                    th = xp.tile([128, 2, 512], F8, tag="x", name=f"xh{j}_{kk}")
                    tl = xp.tile([128, 2, 512], F8, tag="x", name=f"xl{j}_{kk}")
                    nc.sync.dma_start(out=th, in_=x8h[j, kk])
                    nc.sync.dma_start(out=tl, in_=x8l[j, kk])
                    xh.append(th)
                    xl.append(tl)

                # ---- Q/K projections (3-pass fp8 DoubleRow) ----
                for w8, dst in ((wq, qt), (wk, kt)):
                    for es in range(4):
                        esl = slice(es * 128, (es + 1) * 128)
                        ps = pp.tile([128, 512], F32, tag="pp",
                                     name=f"qk{j}{es}")
                        idx = 0
                        for wpart, xpart in ((0, xh), (1, xh), (0, xl)):
                            for kk in range(NKK):
                                nc.tensor.matmul(ps, w8[wpart][:, kk, :, esl],
                                                 xpart[kk], start=(idx == 0),
                                                 stop=(idx == 11),
                                                 perf_mode=DR)
                                idx += 1
                        nc.vector.tensor_copy(dst[es][:, tsl], ps)

                # ---- V projection (3-pass fp8 DoubleRow) ----
                for u in range(4):
                    usl = slice(u * 128, (u + 1) * 128)
                    ps = pp.tile([128, E], F32, tag="pp", name=f"v{j}_{u}")
                    idx = 0
                    for wpart, xpart in ((0, xh), (1, xh), (0, xl)):
                        for kk in range(NKK):
                            nc.tensor.matmul(ps, xpart[kk][:, :, usl],
                                             wv[wpart][:, kk, :, :],
                                             start=(idx == 0),
                                             stop=(idx == 11), perf_mode=DR)
                            idx += 1
                    nc.vector.tensor_copy(
                        vt[:, 4 * j + u, :, 0:DK],
                        ps.rearrange("p (h k) -> p h k", h=HEADS))

                if prev_att is not None:
                    emit_wo(j - 1, prev_att)

                # ---- attention for q-slice j ----
                att_j = [attp.tile([128, 512], F32R, tag="at",
                                   name=f"at{j}_{f}") for f in range(4)]
                npair = 2 * (j + 1)
                for hp in range(4):
                    hA, hB = 2 * hp, 2 * hp + 1
                    pvA = pvp.tile([DK + 1, 512], F32, tag="pv",
                                   name=f"pvA{j}{hp}")
                    pvB = pvp.tile([DK + 1, 512], F32, tag="pv",
                                   name=f"pvB{j}{hp}")
                    for g in range(npair):
                        r0 = 2 * g - 4 * j
                        q0 = min(128 * r0, 256) if r0 > 0 else 0
                        qv = slice(j * 512 + q0, (j + 1) * 512)
                        stA = stp.tile([128, 2, 512], F32, tag="st",
                                       name=f"stA{j}{hp}{g}")
                        stB = stp.tile([128, 2, 512], F32, tag="st",
                                       name=f"stB{j}{hp}{g}")
                        exA = exq.tile([128, 2, 512], BF16, tag="ex",
                                       name=f"exA{j}{hp}{g}")
                        exB = exq.tile([128, 2, 512], BF16, tag="ex",
                                       name=f"exB{j}{hp}{g}")
                        for u in range(2):
                            i = 2 * g + u
                            ssl = slice(i * 128, (i + 1) * 128)
                            nc.tensor.matmul(stA[:, u, q0:], kt[hp][0:64, ssl],
                                             qt[hp][0:64, qv], start=True,
                                             stop=True, tile_position=(0, 0))
                            nc.tensor.matmul(stB[:, u, q0:], kt[hp][64:128, ssl],
                                             qt[hp][64:128, qv], start=True,
                                             stop=True, tile_position=(64, 0))
                        nc.scalar.activation(exA[:, :, q0:], stA[:, :, q0:],
                                             AF.Exp, scale=SCALE, bias=ebias)
                        nc.scalar.activation(exB[:, :, q0:], stB[:, :, q0:],
                                             AF.Exp, scale=SCALE, bias=ebias)
                        if r0 >= 0:
                            # zero E where s > q (sub-diagonal triangle and
                            # the causally invalid columns kept by the N>=256
                            # clamp)
                            for ex in (exA, exB):
                                nc.gpsimd.affine_select(
                                    out=ex[:, :, q0:], in_=ex[:, :, q0:],
                                    pattern=[[-128, 2], [1, 512 - q0]],
                                    compare_op=mybir.AluOpType.is_ge,
                                    fill=0.0, base=q0 - 128 * r0,
                                    channel_multiplier=-1)
                        for u in range(2):
                            i = 2 * g + u
                            nc.tensor.matmul(pvA[:, q0:], vt[:, i, hA, :],
                                             exA[:, u, q0:],
                                             start=(g == 0 and u == 0),
                                             stop=(g == npair - 1 and u == 1))
                            nc.tensor.matmul(pvB[:, q0:], vt[:, i, hB, :],
                                             exB[:, u, q0:],
                                             start=(g == 0 and u == 0),
                                             stop=(g == npair - 1 and u == 1))
                    for pvx, h in ((pvA, hA), (pvB, hB)):
                        rz = nrmp.tile([1, 512], F32, tag="rz",
                                       name=f"rz{j}{h}")
                        bz = nrmp.tile([DK, 512], F32, tag="bz",
                                       name=f"bz{j}{h}")
                        nc.vector.reciprocal(rz, pvx[DK:DK + 1, :])
                        nc.gpsimd.partition_broadcast(bz, rz)
                        f, ho = divmod(h, 2)
                        p0 = 64 * ho
                        nc.vector.tensor_mul(att_j[f][p0:p0 + 64, :],
                                             pvx[0:DK, :], bz)

                prev_att = att_j
            emit_wo(NQ - 1, prev_att)



def build_nc(reps=1):
    nc = bacc.Bacc(None, target_bir_lowering=False, debug=False)
    x8h = nc.dram_tensor("x8h", [NQ, NKK, 128, 2, 512], F8,
                         kind="ExternalInput")
    x8l = nc.dram_tensor("x8l", [NQ, NKK, 128, 2, 512], F8,
                         kind="ExternalInput")
    names = ["wq8h", "wq8l", "wk8h", "wk8l", "wv8h", "wv8l"]
    w8 = [nc.dram_tensor(n, [128, NKK, 2, E], F8, kind="ExternalInput")
          for n in names]
    woT = nc.dram_tensor("woT", [128, 4, D_MODEL], F32R, kind="ExternalInput")
    out = nc.dram_tensor("out", [S, D_MODEL], F32, kind="ExternalOutput")
    aps = (x8h.ap(), x8l.ap(), *[w.ap() for w in w8], woT.ap(), out.ap())
    with tile.TileContext(nc) as tc:
        if reps == 1:
            _emit(tc, *aps)
        else:
            with tc.For_i(0, reps, 1):
                _emit(tc, *aps)
    nc.compile()
    return nc


def _x8_pack(xt):
    """[D, S] -> [j, kk, p, ud, t] with d = kk*256 + ud*128 + p."""
    a = xt.reshape(NKK, 2, 128, NQ, 512).transpose(3, 0, 2, 1, 4)
    return np.ascontiguousarray(a)


def _w_pack(w):
    """[512 e, 1024 d] -> [p, kk, ud, e] with d = kk*256 + ud*128 + p."""
    arr = w.T.reshape(NKK, 2, 128, E).transpose(2, 0, 1, 3)
    return np.ascontiguousarray(arr.astype(E4))


def make_in_maps(x, W_Q, W_K, W_V, W_O):
    in_maps = []
    xh_b = []
    xl_b = []
    for b in range(B):
        xt = np.ascontiguousarray(x[b].T)          # [D, S]
        xh = xt.astype(E4)
        xl = (xt - xh.astype(np.float32)).astype(E4)
        xh_b.append(_x8_pack(xh))
        xl_b.append(_x8_pack(xl))
    for c in range(8):
        b, g = divmod(c, 2)
        sl = slice(g * E, (g + 1) * E)
        m = {"x8h": xh_b[b], "x8l": xl_b[b]}
        for nm, W in (("wq8", W_Q), ("wk8", W_K), ("wv8", W_V)):
            ws = (W[sl] * W_SCALE).astype(np.float32)
            hi = ws.astype(E4)
            lo = (ws - hi.astype(np.float32)).astype(np.float32)
            m[nm + "h"] = _w_pack(hi.astype(np.float32))
            m[nm + "l"] = _w_pack(lo)
        m["woT"] = np.ascontiguousarray(
            (W_O[:, sl].T / W_SCALE).reshape(4, 128, D_MODEL)
            .transpose(1, 0, 2).astype(np.float32))
        in_maps.append(m)
    return in_maps


_NC_CACHE = None


def kernel(x, W_Q, W_K, W_V, W_O, _trace=False):
    global _NC_CACHE
    if _NC_CACHE is None:
        _NC_CACHE = build_nc()
    nc = _NC_CACHE
    in_maps = make_in_maps(x, W_Q, W_K, W_V, W_O)
    r = run_bass_kernel_spmd(nc, in_maps, list(range(8)), trace=_trace)
    kernel.last_result = r
    out = np.empty((B, S, D_MODEL), np.float32)
    for b in range(B):
        out[b] = r.results[2 * b]["out"] + r.results[2 * b + 1]["out"]
    return out


# revision 10
# speedup vs baseline: 1.2211x; 1.1521x over previous
"""Multi-head causal self-attention (B=4, S=2048, D=1024, H=16) on 8 trn2 cores.

Sharding: core c = (batch b = c//2, head-group g = c%2 of 8 heads).
Each core computes Q/K/V projections for its 8 heads over its batch's 2048
tokens, causal attention, and a PARTIAL output projection over its 512
feature dims. The host adds the two partial [2048, 1024] outputs per batch.
No on-device collectives.

vs the original f32r kernel:
  - Q/K/V projections run as 3-pass fp8e4m3 DoubleRow (W,x split hi+lo on
    host, lo*lo term dropped; W pre-scaled x32 to clear e4m3 subnormals,
    compensated in the exp scale / W_O). 256-contraction per instruction at
    0.5 cyc/row: 25% fewer PE cycles than f32r, error ~1e-3.
  - Scores, exp output E, and PV run in bf16 (same PE rate as f32r, half
    the SBUF, 2x DVE copies).
  - The causal mask costs ZERO PE work: score tiles are trimmed to the
    pair-level causal window (clamped to N>=256 to keep full PE rate) and
    the sub-diagonal triangle of E is zeroed by gpsimd affine_select.
  - exp scale folds 1/sqrt(dk), the x32^2 weight scale, and a -1 bias
    (headroom; cancels in the softmax ratio).
  - A ones-column in V makes each PV matmul also emit the softmax
    denominator Z.
  - Projections, attention and W_O are emitted per 512-token q-slice in one
    loop so ACT exp work overlaps PE matmuls across slices.
"""

import ml_dtypes
import numpy as np

import concourse.bass as bass
import concourse.mybir as mybir
import concourse.tile as tile
from concourse import bacc
from concourse.bass_utils import run_bass_kernel_spmd

F32 = mybir.dt.float32
F32R = mybir.dt.float32r
F8 = mybir.dt.float8e4
BF16 = mybir.dt.bfloat16
AF = mybir.ActivationFunctionType
DR = mybir.MatmulPerfMode.DoubleRow
E4 = ml_dtypes.float8_e4m3fn

B = 4
S = 2048
D_MODEL = 1024
E = 512          # feature dims per core (8 heads x 64)
HEADS = 8        # heads per core
DK = 64
NQ = 4           # 512-token q-slices
NKK = 4          # 256-dim d_model double-tiles
NT = 16          # 128-token tiles
W_SCALE = 32.0   # host pre-scale of W_Q/W_K/W_V before fp8 quantization
SCALE = 0.125 / (W_SCALE * W_SCALE)
EBIAS = -1.0     # exp headroom shift; cancels in softmax ratio


def _emit(tc, x8h, x8l, wq8hd, wq8ld, wk8hd, wk8ld, wv8hd, wv8ld, woT, out):
    nc = tc.nc
    with tc.tile_pool(name="singles", bufs=1) as singles, \
         tc.tile_pool(name="wpool", bufs=2) as wpool:
        wq = [wpool.tile([128, NKK, 2, E], F8, tag=f"wq{p}", name=f"wq{p}") for p in range(2)]
        wk = [wpool.tile([128, NKK, 2, E], F8, tag=f"wk{p}", name=f"wk{p}") for p in range(2)]
        wv = [wpool.tile([128, NKK, 2, E], F8, tag=f"wv{p}", name=f"wv{p}") for p in range(2)]
        wot = wpool.tile([128, 4, D_MODEL], BF16, tag="wot", name="wot")
        for t, d in ((wq[0], wq8hd), (wq[1], wq8ld), (wk[0], wk8hd),
                     (wk[1], wk8ld), (wv[0], wv8hd), (wv[1], wv8ld),
                     (wot, woT)):
            nc.sync.dma_start(out=t, in_=d)
        # K/Q with dk on partitions, head pair hp = (2hp, 2hp+1): [128, S]
        qt = [singles.tile([128, S], BF16, name=f"qt{hp}") for hp in range(4)]
        kt = [singles.tile([128, S], BF16, name=f"kt{hp}") for hp in range(4)]
        # V + ones column: [tok 128, s-tile, head, dk+1]
        vt = singles.tile([128, NT, HEADS, DK + 1], BF16, name="vt")
        nc.gpsimd.memset(vt[:, :, :, DK:DK + 1], 1.0)
        ebias = singles.tile([128, 1], F32, name="ebias")
        nc.gpsimd.memset(ebias, EBIAS)

        with (
            tc.tile_pool(name="xp", bufs=16) as xp,
            tc.tile_pool(name="pp", bufs=2, space="PSUM") as pp,
            tc.tile_pool(name="stp", bufs=2, space="PSUM") as stp,
            tc.tile_pool(name="pvp", bufs=2, space="PSUM") as pvp,
            tc.tile_pool(name="exq", bufs=6) as exq,
            tc.tile_pool(name="attp", bufs=8) as attp,
            tc.tile_pool(name="nrmp", bufs=4) as nrmp,
            tc.tile_pool(name="outp", bufs=3) as outp,
        ):
            def emit_wo(jw, att_w):
                for tt in range(4):
                    ot = outp.tile([128, D_MODEL], F32, tag="ot",
                                   name=f"ot{jw}{tt}")
                    ttsl = slice(tt * 128, (tt + 1) * 128)
                    for eo in range(2):
                        pso = pp.tile([128, 512], F32, tag="pp",
                                      name=f"po{jw}{tt}{eo}")
                        for f in range(4):
                            nc.tensor.matmul(pso, att_w[f][:, ttsl],
                                             wot[:, f, eo * 512:(eo + 1) * 512],
                                             start=(f == 0), stop=(f == 3))
                        nc.vector.tensor_copy(ot[:, eo * 512:(eo + 1) * 512],
                                              pso)
                    t0 = jw * 512 + tt * 128
                    nc.sync.dma_start(out=out[t0:t0 + 128, :], in_=ot)

            prev_att = None
            for j in range(NQ):
                tsl = slice(j * 512, (j + 1) * 512)
                xh = []
                xl = []
                for kk in range(NKK):
                    th = xp.tile([128, 2, 512], F8, tag="x", name=f"xh{j}_{kk}")
                    tl = xp.tile([128, 2, 512], F8, tag="x", name=f"xl{j}_{kk}")
                    nc.sync.dma_start(out=th, in_=x8h[j, kk])
                    nc.sync.dma_start(out=tl, in_=x8l[j, kk])
                    xh.append(th)
                    xl.append(tl)

                # ---- Q/K projections (3-pass fp8 DoubleRow) ----
                for w8, dst in ((wq, qt), (wk, kt)):
                    for es in range(4):
                        esl = slice(es * 128, (es + 1) * 128)
                        ps = pp.tile([128, 512], F32, tag="pp",
                                     name=f"qk{j}{es}")
                        idx = 0
                        for wpart, xpart in ((0, xh), (1, xh), (0, xl)):
                            for kk in range(NKK):
                                nc.tensor.matmul(ps, w8[wpart][:, kk, :, esl],
                                                 xpart[kk], start=(idx == 0),
                                                 stop=(idx == 11),
                                                 perf_mode=DR)
                                idx += 1
                        nc.vector.tensor_copy(dst[es][:, tsl], ps)

                # ---- V projection (3-pass fp8 DoubleRow) ----
                for u in range(4):
                    usl = slice(u * 128, (u + 1) * 128)
                    ps = pp.tile([128, E], F32, tag="pp", name=f"v{j}_{u}")
                    idx = 0
                    for wpart, xpart in ((0, xh), (1, xh), (0, xl)):
                        for kk in range(NKK):
                            nc.tensor.matmul(ps, xpart[kk][:, :, usl],
                                             wv[wpart][:, kk, :, :],
                                             start=(idx == 0),
                                             stop=(idx == 11), perf_mode=DR)
                            idx += 1
                    nc.vector.tensor_copy(
                        vt[:, 4 * j + u, :, 0:DK],
                        ps.rearrange("p (h k) -> p h k", h=HEADS))

                if prev_att is not None:
                    emit_wo(j - 1, prev_att)

                # ---- attention for q-slice j ----
                att_j = [attp.tile([128, 512], BF16, tag="at",
                                   name=f"at{j}_{f}") for f in range(4)]
                npair = 2 * (j + 1)
                for hp in range(4):
                    hA, hB = 2 * hp, 2 * hp + 1
                    pvA = pvp.tile([DK + 1, 512], F32, tag="pv",
                                   name=f"pvA{j}{hp}")
                    pvB = pvp.tile([DK + 1, 512], F32, tag="pv",
                                   name=f"pvB{j}{hp}")
                    for g in range(npair):
                        r0 = 2 * g - 4 * j
                        q0 = min(128 * r0, 256) if r0 > 0 else 0
                        qv = slice(j * 512 + q0, (j + 1) * 512)
                        stA = stp.tile([128, 2, 512], F32, tag="st",
                                       name=f"stA{j}{hp}{g}")
                        stB = stp.tile([128, 2, 512], F32, tag="st",
                                       name=f"stB{j}{hp}{g}")
                        exA = exq.tile([128, 2, 512], BF16, tag="ex",
                                       name=f"exA{j}{hp}{g}")
                        exB = exq.tile([128, 2, 512], BF16, tag="ex",
                                       name=f"exB{j}{hp}{g}")
                        for u in range(2):
                            i = 2 * g + u
                            ssl = slice(i * 128, (i + 1) * 128)
                            nc.tensor.matmul(stA[:, u, q0:], kt[hp][0:64, ssl],
                                             qt[hp][0:64, qv], start=True,
                                             stop=True, tile_position=(0, 0))
                            nc.tensor.matmul(stB[:, u, q0:], kt[hp][64:128, ssl],
                                             qt[hp][64:128, qv], start=True,
                                             stop=True, tile_position=(64, 0))
                        nc.scalar.activation(exA[:, :, q0:], stA[:, :, q0:],
                                             AF.Exp, scale=SCALE, bias=ebias)
                        nc.scalar.activation(exB[:, :, q0:], stB[:, :, q0:],
                                             AF.Exp, scale=SCALE, bias=ebias)
                        if r0 >= 0:
                            # zero E where s > q (sub-diagonal triangle and
                            # the causally invalid columns kept by the N>=256
                            # clamp)
                            for ex in (exA, exB):
                                nc.gpsimd.affine_select(
                                    out=ex[:, :, q0:], in_=ex[:, :, q0:],
                                    pattern=[[-128, 2], [1, 512 - q0]],
                                    compare_op=mybir.AluOpType.is_ge,
                                    fill=0.0, base=q0 - 128 * r0,
                                    channel_multiplier=-1)
                        for u in range(2):
                            i = 2 * g + u
                            nc.tensor.matmul(pvA[:, q0:], vt[:, i, hA, :],
                                             exA[:, u, q0:],
                                             start=(g == 0 and u == 0),
                                             stop=(g == npair - 1 and u == 1))
                            nc.tensor.matmul(pvB[:, q0:], vt[:, i, hB, :],
                                             exB[:, u, q0:],
                                             start=(g == 0 and u == 0),
                                             stop=(g == npair - 1 and u == 1))
                    for pvx, h in ((pvA, hA), (pvB, hB)):
                        rz = nrmp.tile([1, 512], F32, tag="rz",
                                       name=f"rz{j}{h}")
                        bz = nrmp.tile([DK, 512], F32, tag="bz",
                                       name=f"bz{j}{h}")
                        nc.vector.reciprocal(rz, pvx[DK:DK + 1, :])
                        nc.gpsimd.partition_broadcast(bz, rz)
                        f, ho = divmod(h, 2)
                        p0 = 64 * ho
                        nc.vector.tensor_mul(att_j[f][p0:p0 + 64, :],
                                             pvx[0:DK, :], bz)

                prev_att = att_j
            emit_wo(NQ - 1, prev_att)



def build_nc(reps=1):
    nc = bacc.Bacc(None, target_bir_lowering=False, debug=False)
    x8h = nc.dram_tensor("x8h", [NQ, NKK, 128, 2, 512], F8,
                         kind="ExternalInput")
    x8l = nc.dram_tensor("x8l", [NQ, NKK, 128, 2, 512], F8,
                         kind="ExternalInput")
    names = ["wq8h", "wq8l", "wk8h", "wk8l", "wv8h", "wv8l"]
    w8 = [nc.dram_tensor(n, [128, NKK, 2, E], F8, kind="ExternalInput")
          for n in names]
    woT = nc.dram_tensor("woT", [128, 4, D_MODEL], BF16, kind="ExternalInput")
    out = nc.dram_tensor("out", [S, D_MODEL], F32, kind="ExternalOutput")
    aps = (x8h.ap(), x8l.ap(), *[w.ap() for w in w8], woT.ap(), out.ap())
    with tile.TileContext(nc) as tc:
        if reps == 1:
            _emit(tc, *aps)
        else:
            with tc.For_i(0, reps, 1):
                _emit(tc, *aps)
    nc.compile()
    return nc


def _x8_pack(xt):
    """[D, S] -> [j, kk, p, ud, t] with d = kk*256 + ud*128 + p."""
    a = xt.reshape(NKK, 2, 128, NQ, 512).transpose(3, 0, 2, 1, 4)
    return np.ascontiguousarray(a)


def _w_pack(w):
    """[512 e, 1024 d] -> [p, kk, ud, e] with d = kk*256 + ud*128 + p."""
    arr = w.T.reshape(NKK, 2, 128, E).transpose(2, 0, 1, 3)
    return np.ascontiguousarray(arr.astype(E4))


def make_in_maps(x, W_Q, W_K, W_V, W_O):
    in_maps = []
    xh_b = []
    xl_b = []
    for b in range(B):
        xt = np.ascontiguousarray(x[b].T)          # [D, S]
        xh = xt.astype(E4)
        xl = (xt - xh.astype(np.float32)).astype(E4)
        xh_b.append(_x8_pack(xh))
        xl_b.append(_x8_pack(xl))
    for c in range(8):
        b, g = divmod(c, 2)
        sl = slice(g * E, (g + 1) * E)
        m = {"x8h": xh_b[b], "x8l": xl_b[b]}
        for nm, W in (("wq8", W_Q), ("wk8", W_K), ("wv8", W_V)):
            ws = (W[sl] * W_SCALE).astype(np.float32)
            hi = ws.astype(E4)
            lo = (ws - hi.astype(np.float32)).astype(np.float32)
            m[nm + "h"] = _w_pack(hi.astype(np.float32))
            m[nm + "l"] = _w_pack(lo)
        m["woT"] = np.ascontiguousarray(
            (W_O[:, sl].T / W_SCALE).reshape(4, 128, D_MODEL)
            .transpose(1, 0, 2).astype(ml_dtypes.bfloat16))
        in_maps.append(m)
    return in_maps


_NC_CACHE = None


def kernel(x, W_Q, W_K, W_V, W_O, _trace=False):
    global _NC_CACHE
    if _NC_CACHE is None:
        _NC_CACHE = build_nc()
    nc = _NC_CACHE
    in_maps = make_in_maps(x, W_Q, W_K, W_V, W_O)
    r = run_bass_kernel_spmd(nc, in_maps, list(range(8)), trace=_trace)
    kernel.last_result = r
    out = np.empty((B, S, D_MODEL), np.float32)
    for b in range(B):
        out[b] = r.results[2 * b]["out"] + r.results[2 * b + 1]["out"]
    return out


# revision 11
# speedup vs baseline: 1.3874x; 1.1361x over previous
"""Multi-head causal self-attention (B=4, S=2048, D=1024, H=16) on 8 trn2 cores.

Sharding: core c = (batch b = c//2, head-group g = c%2 of 8 heads).
Each core computes Q/K/V projections for its 8 heads over its batch's 2048
tokens, causal attention, and a PARTIAL output projection over its 512
feature dims. The host adds the two partial [2048, 1024] outputs per batch.
No on-device collectives.

All matmul operands are bf16 (error ~3e-3 vs the 2e-2 gate); PSUM stays
f32. Design is driven by measured per-instruction HW costs (~90-270ns fixed
per matmul, K=64 matmuls 1.6x slower than K=128):
  - Projections: single-pass bf16, 8-matmul accumulation chains (K=128).
  - Scores: one K=128 matmul per (head, s-tile): K is zero-padded by
    keeping per-head Q tiles (qtA/qtB) with the OTHER head's 64 rows
    pinned to zero, so no K=64 tile-mode penalty and no extra matmuls.
  - The causal mask costs zero PE work: score tiles are trimmed to the
    pair-level causal window and the sub-diagonal triangle of E is zeroed
    by gpsimd affine_select on the bf16 exp output.
  - A ones-column in V makes each PV matmul also emit the softmax
    denominator Z; PV accumulates in bf16 (full-rate accumulation).
  - W_O runs bf16 on [att * (1/Z)] tiles.
  - Projections(j), W_O(j-1), attention(j) are emitted in one loop so the
    PE always has ready work at slice boundaries and ACT exp overlaps PE.
Constant tiles (q zero-halves, V ones column, exp bias) live outside the
rep loop.
"""

import ml_dtypes
import numpy as np

import concourse.bass as bass
import concourse.mybir as mybir
import concourse.tile as tile
from concourse import bacc
from concourse.bass_utils import run_bass_kernel_spmd

F32 = mybir.dt.float32
BF16 = mybir.dt.bfloat16
AF = mybir.ActivationFunctionType
NPBF = ml_dtypes.bfloat16

B = 4
S = 2048
D_MODEL = 1024
E = 512          # feature dims per core (8 heads x 64)
HEADS = 8
DK = 64
NQ = 4           # 512-token q-slices
ND = 8           # 128-dim d_model tiles
NT = 16          # 128-token tiles
SCALE = 0.125
EBIAS = -1.0     # exp headroom shift; cancels in softmax ratio


def _persist(tc, pool):
    nc = tc.nc
    t = {}
    # Q tiles with the other head's rows pinned to zero (K=128 scores)
    t["qtA"] = [pool.tile([128, S], BF16, name=f"qtA{hp}") for hp in range(4)]
    t["qtB"] = [pool.tile([128, S], BF16, name=f"qtB{hp}") for hp in range(4)]
    t["kt"] = [pool.tile([128, S], BF16, name=f"kt{hp}") for hp in range(4)]
    t["vt"] = pool.tile([128, NT, HEADS, DK + 1], BF16, name="vt")
    t["ebias"] = pool.tile([128, 1], F32, name="ebias")
    for hp in range(4):
        nc.gpsimd.memset(t["qtA"][hp][64:128, :], 0.0)
        nc.gpsimd.memset(t["qtB"][hp][0:64, :], 0.0)
    nc.gpsimd.memset(t["vt"][:, :, :, DK:DK + 1], 1.0)
    nc.gpsimd.memset(t["ebias"], EBIAS)
    return t


def _emit(tc, t, xT, wqT, wkT, wvT, woT, out):
    nc = tc.nc
    qtA, qtB, kt = t["qtA"], t["qtB"], t["kt"]
    vt, ebias = t["vt"], t["ebias"]
    with (
        tc.tile_pool(name="wpool", bufs=2) as wpool,
        tc.tile_pool(name="xp", bufs=16) as xp,
        tc.tile_pool(name="pp", bufs=2, space="PSUM") as pp,
        tc.tile_pool(name="stp", bufs=2, space="PSUM") as stp,
        tc.tile_pool(name="pvp", bufs=2, space="PSUM") as pvp,
        tc.tile_pool(name="exq", bufs=6) as exq,
        tc.tile_pool(name="attp", bufs=8) as attp,
        tc.tile_pool(name="nrmp", bufs=4) as nrmp,
        tc.tile_pool(name="outp", bufs=3) as outp,
    ):
        wq = wpool.tile([128, ND, E], BF16, tag="wq", name="wq")
        wk = wpool.tile([128, ND, E], BF16, tag="wk", name="wk")
        wv = wpool.tile([128, ND, E], BF16, tag="wv", name="wv")
        wot = wpool.tile([128, 4, D_MODEL], BF16, tag="wot", name="wot")
        for tt, dd in ((wq, wqT), (wk, wkT), (wv, wvT), (wot, woT)):
            nc.sync.dma_start(out=tt, in_=dd)

        def emit_wo(jw, att_w):
            for tt in range(4):
                ot = outp.tile([128, D_MODEL], F32, tag="ot", name=f"ot{jw}{tt}")
                ttsl = slice(tt * 128, (tt + 1) * 128)
                for eo in range(2):
                    pso = pp.tile([128, 512], F32, tag="pp",
                                  name=f"po{jw}{tt}{eo}")
                    for f in range(4):
                        nc.tensor.matmul(pso, att_w[f][:, ttsl],
                                         wot[:, f, eo * 512:(eo + 1) * 512],
                                         start=(f == 0), stop=(f == 3))
                    nc.vector.tensor_copy(ot[:, eo * 512:(eo + 1) * 512], pso)
                t0 = jw * 512 + tt * 128
                nc.sync.dma_start(out=out[t0:t0 + 128, :], in_=ot)

        prev_att = None
        for j in range(NQ):
            tsl = slice(j * 512, (j + 1) * 512)
            xc = []
            for d in range(ND):
                ck = xp.tile([128, 512], BF16, tag="x", name=f"x{j}_{d}")
                nc.sync.dma_start(out=ck, in_=xT[j, d])
                xc.append(ck)

            # ---- Q/K projections (bf16, 8-chains) ----
            for w8, dst in ((wq, "q"), (wk, kt)):
                for es in range(4):
                    esl = slice(es * 128, (es + 1) * 128)
                    ps = pp.tile([128, 512], F32, tag="pp", name=f"qk{j}{es}")
                    for d in range(ND):
                        nc.tensor.matmul(ps, w8[:, d, esl], xc[d],
                                         start=(d == 0), stop=(d == ND - 1))
                    if dst == "q":
                        nc.vector.tensor_copy(qtA[es][0:64, tsl], ps[0:64, :])
                        nc.vector.tensor_copy(qtB[es][64:128, tsl],
                                              ps[64:128, :])
                    else:
                        nc.vector.tensor_copy(dst[es][:, tsl], ps)

            # ---- V projection (bf16, 8-chains) ----
            for u in range(4):
                usl = slice(u * 128, (u + 1) * 128)
                ps = pp.tile([128, E], F32, tag="pp", name=f"v{j}_{u}")
                for d in range(ND):
                    nc.tensor.matmul(ps, xc[d][:, usl], wv[:, d, :],
                                     start=(d == 0), stop=(d == ND - 1))
                nc.vector.tensor_copy(
                    vt[:, 4 * j + u, :, 0:DK],
                    ps.rearrange("p (h k) -> p h k", h=HEADS))

            if prev_att is not None:
                emit_wo(j - 1, prev_att)

            # ---- attention for q-slice j ----
            att_j = [attp.tile([128, 512], BF16, tag="at",
                               name=f"at{j}_{f}") for f in range(4)]
            npair = 2 * (j + 1)
            for hp in range(4):
                hA, hB = 2 * hp, 2 * hp + 1
                pvA = pvp.tile([DK + 1, 512], F32, tag="pv", name=f"pvA{j}{hp}")
                pvB = pvp.tile([DK + 1, 512], F32, tag="pv", name=f"pvB{j}{hp}")
                for g in range(npair):
                    r0 = 2 * g - 4 * j
                    q0 = 128 * r0 if r0 > 0 else 0
                    qv = slice(j * 512 + q0, (j + 1) * 512)
                    stA = stp.tile([128, 2, 512], F32, tag="st",
                                   name=f"stA{j}{hp}{g}")
                    stB = stp.tile([128, 2, 512], F32, tag="st",
                                   name=f"stB{j}{hp}{g}")
                    exA = exq.tile([128, 2, 512], BF16, tag="ex",
                                   name=f"exA{j}{hp}{g}")
                    exB = exq.tile([128, 2, 512], BF16, tag="ex",
                                   name=f"exB{j}{hp}{g}")
                    for u in range(2):
                        i = 2 * g + u
                        ssl = slice(i * 128, (i + 1) * 128)
                        nc.tensor.matmul(stA[:, u, q0:], kt[hp][:, ssl],
                                         qtA[hp][:, qv], start=True, stop=True)
                        nc.tensor.matmul(stB[:, u, q0:], kt[hp][:, ssl],
                                         qtB[hp][:, qv], start=True, stop=True)
                    nc.scalar.activation(exA[:, :, q0:], stA[:, :, q0:],
                                         AF.Exp, scale=SCALE, bias=ebias)
                    nc.scalar.activation(exB[:, :, q0:], stB[:, :, q0:],
                                         AF.Exp, scale=SCALE, bias=ebias)
                    if r0 >= 0:
                        # zero E where s > q (sub-diagonal triangle plus the
                        # causally invalid block computed for u=1)
                        for ex in (exA, exB):
                            nc.gpsimd.affine_select(
                                out=ex[:, :, q0:], in_=ex[:, :, q0:],
                                pattern=[[-128, 2], [1, 512 - q0]],
                                compare_op=mybir.AluOpType.is_ge,
                                fill=0.0, base=0, channel_multiplier=-1)
                    for u in range(2):
                        i = 2 * g + u
                        nc.tensor.matmul(pvA[:, q0:], vt[:, i, hA, :],
                                         exA[:, u, q0:],
                                         start=(g == 0 and u == 0),
                                         stop=(g == npair - 1 and u == 1))
                        nc.tensor.matmul(pvB[:, q0:], vt[:, i, hB, :],
                                         exB[:, u, q0:],
                                         start=(g == 0 and u == 0),
                                         stop=(g == npair - 1 and u == 1))
                for pvx, h in ((pvA, hA), (pvB, hB)):
                    rz = nrmp.tile([1, 512], F32, tag="rz", name=f"rz{j}{h}")
                    bz = nrmp.tile([DK, 512], F32, tag="bz", name=f"bz{j}{h}")
                    nc.vector.reciprocal(rz, pvx[DK:DK + 1, :])
                    nc.gpsimd.partition_broadcast(bz, rz)
                    f, ho = divmod(h, 2)
                    p0 = 64 * ho
                    nc.vector.tensor_mul(att_j[f][p0:p0 + 64, :],
                                         pvx[0:DK, :], bz)

            prev_att = att_j
        emit_wo(NQ - 1, prev_att)


def build_nc(reps=1):
    nc = bacc.Bacc(None, target_bir_lowering=False, debug=False)
    xT = nc.dram_tensor("xT", [NQ, ND, 128, 512], BF16, kind="ExternalInput")
    wqT = nc.dram_tensor("wqT", [128, ND, E], BF16, kind="ExternalInput")
    wkT = nc.dram_tensor("wkT", [128, ND, E], BF16, kind="ExternalInput")
    wvT = nc.dram_tensor("wvT", [128, ND, E], BF16, kind="ExternalInput")
    woT = nc.dram_tensor("woT", [128, 4, D_MODEL], BF16, kind="ExternalInput")
    out = nc.dram_tensor("out", [S, D_MODEL], F32, kind="ExternalOutput")
    aps = (xT.ap(), wqT.ap(), wkT.ap(), wvT.ap(), woT.ap(), out.ap())
    with tile.TileContext(nc) as tc:
        with tc.tile_pool(name="persist", bufs=1) as pool:
            t = _persist(tc, pool)
            if reps == 1:
                _emit(tc, t, *aps)
            else:
                with tc.For_i(0, reps, 1):
                    _emit(tc, t, *aps)
    nc.compile()
    return nc


def make_in_maps(x, W_Q, W_K, W_V, W_O):
    in_maps = []
    xT_b = []
    for b in range(B):
        xt = x[b].T.astype(NPBF)  # [D, S]
        xT_b.append(np.ascontiguousarray(
            xt.reshape(ND, 128, NQ, 512).transpose(2, 0, 1, 3)))
    for c in range(8):
        b, g = divmod(c, 2)
        sl = slice(g * E, (g + 1) * E)
        def wtile(w):  # [D, E] -> [128, ND, E]
            return np.ascontiguousarray(
                w.reshape(ND, 128, -1).transpose(1, 0, 2).astype(NPBF))
        in_maps.append({
            "xT": xT_b[b],
            "wqT": wtile(W_Q[sl, :].T),
            "wkT": wtile(W_K[sl, :].T),
            "wvT": wtile(W_V[sl, :].T),
            "woT": np.ascontiguousarray(
                W_O[:, sl].T.reshape(4, 128, D_MODEL)
                .transpose(1, 0, 2).astype(NPBF)),
        })
    return in_maps


_NC_CACHE = None


def kernel(x, W_Q, W_K, W_V, W_O, _trace=False):
    global _NC_CACHE
    if _NC_CACHE is None:
        _NC_CACHE = build_nc()
    nc = _NC_CACHE
    in_maps = make_in_maps(x, W_Q, W_K, W_V, W_O)
    r = run_bass_kernel_spmd(nc, in_maps, list(range(8)), trace=_trace)
    kernel.last_result = r
    out = np.empty((B, S, D_MODEL), np.float32)
    for b in range(B):
        out[b] = r.results[2 * b]["out"] + r.results[2 * b + 1]["out"]
    return out


# revision 12
# speedup vs baseline: 1.3883x; 1.0007x over previous
"""Multi-head causal self-attention (B=4, S=2048, D=1024, H=16) on 8 trn2 cores.

Sharding: core c = (batch b = c//2, head-group g = c%2 of 8 heads).
Each core computes Q/K/V projections for its 8 heads over its batch's 2048
tokens, causal attention, and a PARTIAL output projection over its 512
feature dims. The host adds the two partial [2048, 1024] outputs per batch.
No on-device collectives.

All matmul operands are bf16 (error ~3e-3 vs the 2e-2 gate); PSUM stays
f32. Design is driven by measured per-instruction HW costs (~90-270ns fixed
per matmul, K=64 matmuls 1.6x slower than K=128):
  - Projections: single-pass bf16, 8-matmul accumulation chains (K=128).
  - Scores: one K=128 matmul per (head, s-tile): K is zero-padded by
    keeping per-head Q tiles (qtA/qtB) with the OTHER head's 64 rows
    pinned to zero, so no K=64 tile-mode penalty and no extra matmuls.
  - The causal mask costs zero PE work: score tiles are trimmed to the
    pair-level causal window and the sub-diagonal triangle of E is zeroed
    by gpsimd affine_select on the bf16 exp output.
  - A ones-column in V makes each PV matmul also emit the softmax
    denominator Z; PV accumulates in bf16 (full-rate accumulation).
  - W_O runs bf16 on [att * (1/Z)] tiles.
  - Projections(j), W_O(j-1), attention(j) are emitted in one loop so the
    PE always has ready work at slice boundaries and ACT exp overlaps PE.
Constant tiles (q zero-halves, V ones column, exp bias) live outside the
rep loop.
"""

import ml_dtypes
import numpy as np

import concourse.bass as bass
import concourse.mybir as mybir
import concourse.tile as tile
from concourse import bacc
from concourse.bass_utils import run_bass_kernel_spmd

F32 = mybir.dt.float32
BF16 = mybir.dt.bfloat16
AF = mybir.ActivationFunctionType
NPBF = ml_dtypes.bfloat16

B = 4
S = 2048
D_MODEL = 1024
E = 512          # feature dims per core (8 heads x 64)
HEADS = 8
DK = 64
NQ = 4           # 512-token q-slices
ND = 8           # 128-dim d_model tiles
NT = 16          # 128-token tiles
SCALE = 0.125
EBIAS = -1.0     # exp headroom shift; cancels in softmax ratio


def _persist(tc, pool):
    nc = tc.nc
    t = {}
    # Q tiles with the other head's rows pinned to zero (K=128 scores)
    t["qtA"] = [pool.tile([128, S], BF16, name=f"qtA{hp}") for hp in range(4)]
    t["qtB"] = [pool.tile([128, S], BF16, name=f"qtB{hp}") for hp in range(4)]
    t["kt"] = [pool.tile([128, S], BF16, name=f"kt{hp}") for hp in range(4)]
    t["vt"] = pool.tile([128, NT, HEADS, DK + 1], BF16, name="vt")
    t["ebias"] = pool.tile([128, 1], F32, name="ebias")
    for hp in range(4):
        nc.gpsimd.memset(t["qtA"][hp][64:128, :], 0.0)
        nc.gpsimd.memset(t["qtB"][hp][0:64, :], 0.0)
    nc.gpsimd.memset(t["vt"][:, :, :, DK:DK + 1], 1.0)
    nc.gpsimd.memset(t["ebias"], EBIAS)
    return t


def _emit(tc, t, xT, wqT, wkT, wvT, woT, out):
    nc = tc.nc
    qtA, qtB, kt = t["qtA"], t["qtB"], t["kt"]
    vt, ebias = t["vt"], t["ebias"]
    with (
        tc.tile_pool(name="wpool", bufs=2) as wpool,
        tc.tile_pool(name="xp", bufs=16) as xp,
        tc.tile_pool(name="stp", bufs=3, space="PSUM") as stp,
        tc.tile_pool(name="pvp", bufs=2, space="PSUM") as pvp,
        tc.tile_pool(name="exq", bufs=6) as exq,
        tc.tile_pool(name="attp", bufs=8) as attp,
        tc.tile_pool(name="nrmp", bufs=4) as nrmp,
        tc.tile_pool(name="outp", bufs=3) as outp,
    ):
        wq = wpool.tile([128, ND, E], BF16, tag="wq", name="wq")
        wk = wpool.tile([128, ND, E], BF16, tag="wk", name="wk")
        wv = wpool.tile([128, ND, E], BF16, tag="wv", name="wv")
        wot = wpool.tile([128, 4, D_MODEL], BF16, tag="wot", name="wot")
        for tt, dd in ((wq, wqT), (wk, wkT), (wv, wvT), (wot, woT)):
            nc.sync.dma_start(out=tt, in_=dd)

        def emit_wo(jw, att_w):
            for tt in range(4):
                ot = outp.tile([128, D_MODEL], F32, tag="ot", name=f"ot{jw}{tt}")
                ttsl = slice(tt * 128, (tt + 1) * 128)
                for eo in range(2):
                    pso = stp.tile([128, 512], F32, tag="st",
                                   name=f"po{jw}{tt}{eo}")
                    for f in range(4):
                        nc.tensor.matmul(pso, att_w[f][:, ttsl],
                                         wot[:, f, eo * 512:(eo + 1) * 512],
                                         start=(f == 0), stop=(f == 3))
                    nc.vector.tensor_copy(ot[:, eo * 512:(eo + 1) * 512], pso)
                t0 = jw * 512 + tt * 128
                nc.sync.dma_start(out=out[t0:t0 + 128, :], in_=ot)

        prev_att = None
        for j in range(NQ):
            tsl = slice(j * 512, (j + 1) * 512)
            xc = []
            for d in range(ND):
                ck = xp.tile([128, 512], BF16, tag="x", name=f"x{j}_{d}")
                nc.sync.dma_start(out=ck, in_=xT[j, d])
                xc.append(ck)

            # ---- Q/K projections (bf16, 8-chains) ----
            for w8, dst in ((wq, "q"), (wk, kt)):
                for es in range(4):
                    esl = slice(es * 128, (es + 1) * 128)
                    ps = stp.tile([128, 512], F32, tag="st", name=f"qk{j}{es}")
                    for d in range(ND):
                        nc.tensor.matmul(ps, w8[:, d, esl], xc[d],
                                         start=(d == 0), stop=(d == ND - 1))
                    if dst == "q":
                        nc.vector.tensor_copy(qtA[es][0:64, tsl], ps[0:64, :])
                        nc.vector.tensor_copy(qtB[es][64:128, tsl],
                                              ps[64:128, :])
                    else:
                        nc.vector.tensor_copy(dst[es][:, tsl], ps)

            # ---- V projection (bf16, 8-chains) ----
            for u in range(4):
                usl = slice(u * 128, (u + 1) * 128)
                ps = stp.tile([128, E], F32, tag="st", name=f"v{j}_{u}")
                for d in range(ND):
                    nc.tensor.matmul(ps, xc[d][:, usl], wv[:, d, :],
                                     start=(d == 0), stop=(d == ND - 1))
                nc.vector.tensor_copy(
                    vt[:, 4 * j + u, :, 0:DK],
                    ps.rearrange("p (h k) -> p h k", h=HEADS))

            if prev_att is not None:
                emit_wo(j - 1, prev_att)

            # ---- attention for q-slice j ----
            att_j = [attp.tile([128, 512], BF16, tag="at",
                               name=f"at{j}_{f}") for f in range(4)]
            npair = 2 * (j + 1)
            for hp in range(4):
                hA, hB = 2 * hp, 2 * hp + 1
                pvA = pvp.tile([DK + 1, 512], F32, tag="pv", name=f"pvA{j}{hp}")
                pvB = pvp.tile([DK + 1, 512], F32, tag="pv", name=f"pvB{j}{hp}")
                for g in range(npair):
                    r0 = 2 * g - 4 * j
                    q0 = 128 * r0 if r0 > 0 else 0
                    qv = slice(j * 512 + q0, (j + 1) * 512)
                    stA = stp.tile([128, 2, 512], F32, tag="st",
                                   name=f"stA{j}{hp}{g}")
                    stB = stp.tile([128, 2, 512], F32, tag="st",
                                   name=f"stB{j}{hp}{g}")
                    exA = exq.tile([128, 2, 512], BF16, tag="ex",
                                   name=f"exA{j}{hp}{g}")
                    exB = exq.tile([128, 2, 512], BF16, tag="ex",
                                   name=f"exB{j}{hp}{g}")
                    for u in range(2):
                        i = 2 * g + u
                        ssl = slice(i * 128, (i + 1) * 128)
                        nc.tensor.matmul(stA[:, u, q0:], kt[hp][:, ssl],
                                         qtA[hp][:, qv], start=True, stop=True)
                        nc.tensor.matmul(stB[:, u, q0:], kt[hp][:, ssl],
                                         qtB[hp][:, qv], start=True, stop=True)
                    nc.scalar.activation(exA[:, :, q0:], stA[:, :, q0:],
                                         AF.Exp, scale=SCALE, bias=ebias)
                    nc.scalar.activation(exB[:, :, q0:], stB[:, :, q0:],
                                         AF.Exp, scale=SCALE, bias=ebias)
                    if r0 >= 0:
                        # zero E where s > q (sub-diagonal triangle plus the
                        # causally invalid block computed for u=1)
                        for ex in (exA, exB):
                            nc.gpsimd.affine_select(
                                out=ex[:, :, q0:], in_=ex[:, :, q0:],
                                pattern=[[-128, 2], [1, 512 - q0]],
                                compare_op=mybir.AluOpType.is_ge,
                                fill=0.0, base=0, channel_multiplier=-1)
                    for u in range(2):
                        i = 2 * g + u
                        nc.tensor.matmul(pvA[:, q0:], vt[:, i, hA, :],
                                         exA[:, u, q0:],
                                         start=(g == 0 and u == 0),
                                         stop=(g == npair - 1 and u == 1))
                        nc.tensor.matmul(pvB[:, q0:], vt[:, i, hB, :],
                                         exB[:, u, q0:],
                                         start=(g == 0 and u == 0),
                                         stop=(g == npair - 1 and u == 1))
                for pvx, h in ((pvA, hA), (pvB, hB)):
                    rz = nrmp.tile([1, 512], F32, tag="rz", name=f"rz{j}{h}")
                    bz = nrmp.tile([DK, 512], F32, tag="bz", name=f"bz{j}{h}")
                    nc.vector.reciprocal(rz, pvx[DK:DK + 1, :])
                    nc.gpsimd.partition_broadcast(bz, rz)
                    f, ho = divmod(h, 2)
                    p0 = 64 * ho
                    nc.vector.tensor_mul(att_j[f][p0:p0 + 64, :],
                                         pvx[0:DK, :], bz)

            prev_att = att_j
        emit_wo(NQ - 1, prev_att)


def build_nc(reps=1):
    nc = bacc.Bacc(None, target_bir_lowering=False, debug=False)
    xT = nc.dram_tensor("xT", [NQ, ND, 128, 512], BF16, kind="ExternalInput")
    wqT = nc.dram_tensor("wqT", [128, ND, E], BF16, kind="ExternalInput")
    wkT = nc.dram_tensor("wkT", [128, ND, E], BF16, kind="ExternalInput")
    wvT = nc.dram_tensor("wvT", [128, ND, E], BF16, kind="ExternalInput")
    woT = nc.dram_tensor("woT", [128, 4, D_MODEL], BF16, kind="ExternalInput")
    out = nc.dram_tensor("out", [S, D_MODEL], F32, kind="ExternalOutput")
    aps = (xT.ap(), wqT.ap(), wkT.ap(), wvT.ap(), woT.ap(), out.ap())
    with tile.TileContext(nc) as tc:
        with tc.tile_pool(name="persist", bufs=1) as pool:
            t = _persist(tc, pool)
            if reps == 1:
                _emit(tc, t, *aps)
            else:
                with tc.For_i(0, reps, 1):
                    _emit(tc, t, *aps)
    nc.compile()
    return nc


def make_in_maps(x, W_Q, W_K, W_V, W_O):
    in_maps = []
    xT_b = []
    for b in range(B):
        xt = x[b].T.astype(NPBF)  # [D, S]
        xT_b.append(np.ascontiguousarray(
            xt.reshape(ND, 128, NQ, 512).transpose(2, 0, 1, 3)))
    for c in range(8):
        b, g = divmod(c, 2)
        sl = slice(g * E, (g + 1) * E)
        def wtile(w):  # [D, E] -> [128, ND, E]
            return np.ascontiguousarray(
                w.reshape(ND, 128, -1).transpose(1, 0, 2).astype(NPBF))
        in_maps.append({
            "xT": xT_b[b],
            "wqT": wtile(W_Q[sl, :].T),
            "wkT": wtile(W_K[sl, :].T),
            "wvT": wtile(W_V[sl, :].T),
            "woT": np.ascontiguousarray(
                W_O[:, sl].T.reshape(4, 128, D_MODEL)
                .transpose(1, 0, 2).astype(NPBF)),
        })
    return in_maps


_NC_CACHE = None


def kernel(x, W_Q, W_K, W_V, W_O, _trace=False):
    global _NC_CACHE
    if _NC_CACHE is None:
        _NC_CACHE = build_nc()
    nc = _NC_CACHE
    in_maps = make_in_maps(x, W_Q, W_K, W_V, W_O)
    r = run_bass_kernel_spmd(nc, in_maps, list(range(8)), trace=_trace)
    kernel.last_result = r
    out = np.empty((B, S, D_MODEL), np.float32)
    for b in range(B):
        out[b] = r.results[2 * b]["out"] + r.results[2 * b + 1]["out"]
    return out
